# revision 4
# baseline (speedup 1.0000x reference)
"""TRN2 Bass kernel for nn_Attention_15590731285136.

Computation (per batch b):
    g      = diag(W) * K[b]                       # [d]
    score  = relu(V[b] @ (g[:,None]*w1) + b1) @ w2 + b2   # [h]
    score  = where(mask[b], MASK_FILL, score)
    alpha  = softmax(score)                        # over h
    out[b] = alpha @ V[b]                          # [d]

Sharding: data-parallel over batch, 8 batches per core on 8 NeuronCores.

Key transformations:
  * The elementwise gate folds into the weight matrix: V*(g) @ w1 = V @ (g[:,None]*w1).
  * w2 is folded into w1's columns by |w2|, with columns permuted so all
    positive-sign columns come first. Then score = rowsum(relu(..)[:, :hp])
    - rowsum(relu(..)[:, hp:]), and ACT's fused relu+accum computes both
    row-sums for free.
  * softmax skips max-subtraction (scores are O(0.1); masked entries use an
    additive -2^32 bias so exp underflows to exactly 0), and normalization is
    deferred to a single [1,512] scale at the end.
  * All matmuls run as float32r (full-rate fp32 mode on the PE).
"""

import numpy as np

B, H, D, HID = 64, 2048, 512, 512
NCORES = 8
BPC = B // NCORES          # batches per core
HT = H // 128              # 16 h-tiles per batch
DC = D // 128              # 4 contraction chunks
NGROUPS = 4                # tok-tile groups of 4 tiles (512 tokens)
MASK_FILL = -2.0**32 + 1.0


def _build(hp, b2val, has_bias):
    import concourse.bass as bass
    import concourse.mybir as mybir
    from concourse import bacc
    from concourse.tile import TileContext
    from concourse.masks import make_identity

    F32 = mybir.dt.float32
    F32R = mybir.dt.float32r
    BF16 = mybir.dt.bfloat16
    ACTF = mybir.ActivationFunctionType

    nc = bacc.Bacc(trn_type="TRN2", num_devices=NCORES)

    Vd = nc.dram_tensor("V", (BPC, H, D), F32, kind="ExternalInput")
    GT = nc.dram_tensor("GT", (BPC, 128, DC), F32, kind="ExternalInput")
    MB = nc.dram_tensor("MB", (BPC, 128, HT), F32, kind="ExternalInput")
    WA = nc.dram_tensor("WA", (D, HID), F32, kind="ExternalInput")
    if has_bias:
        BI = nc.dram_tensor("BI", (1, HID), F32, kind="ExternalInput")
    OUT = nc.dram_tensor("OUT", (BPC, D), F32, kind="ExternalOutput")

    with TileContext(nc) as tc:
        with (
            tc.tile_pool(name="const", bufs=1) as cpool,
            tc.tile_pool(name="v", bufs=2 * HT) as vpool,
            tc.tile_pool(name="vt", bufs=2 * DC) as vtpool,
            tc.tile_pool(name="w12", bufs=2) as wpool,
            tc.tile_pool(name="small", bufs=2) as spool,
            tc.tile_pool(name="scr", bufs=2) as scrpool,
            tc.tile_pool(name="fin", bufs=2) as finpool,
            tc.tile_pool(name="vt_ps", bufs=2, space="PSUM") as vtps,
            tc.tile_pool(name="fc1_ps", bufs=2, space="PSUM") as fc1ps,
            tc.tile_pool(name="tot_ps", bufs=2, space="PSUM") as totps,
            tc.tile_pool(name="acc_ps", bufs=2, space="PSUM") as accps,
        ):
            # ---- one-time constants ----
            ident_f32 = cpool.tile([128, 128], F32, tag="identf")
            ident = cpool.tile([128, 128], F32R, tag="ident")
            make_identity(nc, ident_f32)
            nc.vector.tensor_copy(ident, ident_f32)

            ones_f32 = cpool.tile([128, 1], F32, tag="onesf")
            ones_col = cpool.tile([128, 1], F32R, tag="ones")
            nc.vector.memset(ones_f32, 1.0)
            nc.vector.tensor_copy(ones_col, ones_f32)

            # WA as [128, DC*HID]: chunk c at cols [c*HID, (c+1)*HID)
            wabs = cpool.tile([128, DC * HID], F32, tag="wabs")
            nc.sync.dma_start(
                out=wabs.rearrange("p (c n) -> p c n", c=DC),
                in_=WA.ap().rearrange("(c p) n -> p c n", p=128),
            )
            if has_bias:
                ones_row_f = cpool.tile([1, 128], F32, tag="orf")
                ones_row = cpool.tile([1, 128], F32R, tag="orr")
                nc.vector.memset(ones_row_f, 1.0)
                nc.vector.tensor_copy(ones_row, ones_row_f)
                bias_sb = cpool.tile([1, HID], F32R, tag="bias")
                bias_f = cpool.tile([1, HID], F32, tag="biasf")
                nc.sync.dma_start(out=bias_f, in_=BI.ap())
                nc.vector.tensor_copy(bias_sb, bias_f)

            for bi in range(BPC):
                # ---- per-batch small loads ----
                gcol = spool.tile([128, DC], F32, tag="gcol")
                nc.sync.dma_start(out=gcol, in_=GT.ap()[bi])
                mb = spool.tile([128, HT], F32, tag="mb")
                nc.sync.dma_start(out=mb, in_=MB.ap()[bi])

                # ---- gate the packed weights: W12[d, :] = g[d] * Wabs[d, :] ----
                w12 = wpool.tile([128, DC * HID], F32R, tag="w12")
                for c in range(DC):
                    nc.vector.tensor_scalar_mul(
                        w12[:, c * HID:(c + 1) * HID],
                        wabs[:, c * HID:(c + 1) * HID],
                        gcol[:, c:c + 1],
                    )

                # ---- load V[bi] as 16 tiles [128 tok, 512 d] ----
                vt_tiles = []
                for j in range(HT):
                    vj = vpool.tile([128, D], F32R, tag="v")
                    nc.sync.dma_start(
                        out=vj, in_=Vd.ap()[bi, j * 128:(j + 1) * 128, :].bitcast(F32R)
                    )
                    vt_tiles.append(vj)

                sp = spool.tile([128, HT], F32, tag="sp")
                sn = spool.tile([128, HT], F32, tag="sn")
                if hp == 0:
                    nc.vector.memset(sp, 0.0)
                if hp == HID:
                    nc.vector.memset(sn, 0.0)

                for g in range(NGROUPS):
                    # ---- transpose the group's 4 tok-tiles (d on partitions) ----
                    vts = []
                    for c in range(DC):
                        tp = vtps.tile([128, 512], F32, tag="vtp")
                        for t in range(4):
                            j = g * 4 + t
                            nc.tensor.transpose(
                                tp[:, t * 128:(t + 1) * 128].bitcast(F32R),
                                vt_tiles[j][:, c * 128:(c + 1) * 128],
                                ident,
                            )
                        vs = vtpool.tile([128, 512], F32R, tag="vt")
                        nc.vector.tensor_copy(vs, tp)  # rounds f32 -> f32r
                        vts.append(vs)

                    # ---- fc1 + fused relu/rowsum per tok-tile ----
                    for t in range(4):
                        j = g * 4 + t
                        fc1 = fc1ps.tile([128, HID], F32, tag="fc1")
                        for c in range(DC):
                            nc.tensor.matmul(
                                out=fc1,
                                lhsT=vts[c][:, t * 128:(t + 1) * 128],
                                rhs=w12[:, c * HID:(c + 1) * HID],
                                start=(c == 0),
                                stop=(c == DC - 1) and not has_bias,
                            )
                        if has_bias:
                            nc.tensor.matmul(
                                out=fc1, lhsT=ones_row, rhs=bias_sb,
                                start=False, stop=True,
                            )
                        scr = scrpool.tile([128, HID], BF16, tag="scr")
                        if hp > 0:
                            nc.scalar.activation(
                                out=scr[:, :hp], in_=fc1[:, :hp], func=ACTF.Relu,
                                accum_out=sp[:, j:j + 1],
                            )
                        if hp < HID:
                            nc.scalar.activation(
                                out=scr[:, hp:], in_=fc1[:, hp:], func=ACTF.Relu,
                                accum_out=sn[:, j:j + 1],
                            )

                # ---- scores -> masked -> exp ----
                sc = spool.tile([128, HT], F32, tag="sc")
                nc.vector.tensor_sub(sc, sp, sn)
                scm = spool.tile([128, HT], F32, tag="scm")
                nc.vector.tensor_add(scm, sc, mb)
                alpha = spool.tile([128, HT], F32R, tag="alpha")
                nc.scalar.activation(
                    out=alpha, in_=scm, func=ACTF.Exp, bias=float(b2val),
                )

                # ---- denominator: sum over all tokens ----
                tot = totps.tile([1, HT], F32, tag="tot")
                nc.tensor.matmul(out=tot, lhsT=ones_col, rhs=alpha,
                                 start=True, stop=True)
                tot_sb = finpool.tile([1, 1], F32, tag="tot_sb")
                nc.vector.tensor_reduce(
                    tot_sb, tot, axis=mybir.AxisListType.X,
                    op=mybir.AluOpType.add,
                )
                inv = finpool.tile([1, 1], F32, tag="inv")
                nc.vector.reciprocal(inv, tot_sb)

                # ---- pass 2: acc[1, d] = sum_j alpha[:, j]^T @ V_j ----
                acc = accps.tile([1, D], F32, tag="acc")
                for j in range(HT):
                    nc.tensor.matmul(
                        out=acc,
                        lhsT=alpha[:, j:j + 1],
                        rhs=vt_tiles[j],
                        start=(j == 0),
                        stop=(j == HT - 1),
                    )
                ob = finpool.tile([1, D], F32, tag="ob")
                nc.vector.tensor_scalar_mul(ob, acc, inv)
                nc.sync.dma_start(out=OUT.ap()[bi:bi+1, :], in_=ob)

    nc.finalize()
    return nc


def _prep(K, V, mask, W, w1, b1, w2, b2):
    """Host-side input marshalling (no heavy compute)."""
    K = np.asarray(K, dtype=np.float32)
    V = np.ascontiguousarray(np.asarray(V, dtype=np.float32))
    mask = np.asarray(mask)
    W = np.asarray(W, dtype=np.float32)
    w1 = np.asarray(w1, dtype=np.float32)
    b1 = np.asarray(b1, dtype=np.float32)
    w2 = np.asarray(w2, dtype=np.float32).reshape(-1)
    b2 = np.asarray(b2, dtype=np.float32).reshape(-1)

    g = np.diagonal(W).astype(np.float32) * K          # [B, D]
    pos = w2 >= 0.0
    perm = np.argsort(~pos, kind="stable")             # positives first
    hp = int(pos.sum())
    wabs = (w1[:, perm] * np.abs(w2[perm])[None, :]).astype(np.float32)
    bias12 = (b1[perm] * np.abs(w2[perm])).astype(np.float32)
    has_bias = bool(np.any(bias12 != 0.0))

    # g arranged [B, 128, DC] so chunk c sits in column c (partition-major)
    gt = np.ascontiguousarray(g.reshape(B, DC, 128).transpose(0, 2, 1))
    # additive mask bias [B, 128, HT]: token j*128+p -> [p, j]
    mbias = np.where(mask, np.float32(MASK_FILL), np.float32(0.0)).astype(np.float32)
    mbias = np.ascontiguousarray(mbias.reshape(B, HT, 128).transpose(0, 2, 1))
    return V, gt, mbias, wabs, bias12, has_bias, hp, float(b2[0]) if b2.size else 0.0


def kernel(K, V, mask, W, w1, b1, w2, b2):
    from concourse import bass_utils

    Vc, gt, mbias, wabs, bias12, has_bias, hp, b2val = _prep(
        K, V, mask, W, w1, b1, w2, b2
    )
    nc = _build(hp, b2val, has_bias)

    in_maps = []
    for c in range(NCORES):
        sl = slice(c * BPC, (c + 1) * BPC)
        m = {
            "V": Vc[sl],
            "GT": gt[sl],
            "MB": mbias[sl],
            "WA": wabs,
        }
        if has_bias:
            m["BI"] = bias12.reshape(1, HID)
        in_maps.append(m)

    res = bass_utils.run_bass_kernel_spmd(nc, in_maps, core_ids=list(range(NCORES)))
    out = np.concatenate([res.results[c]["OUT"] for c in range(NCORES)], axis=0)
    return out.astype(np.float32)
